# revision 25
# baseline (speedup 1.0000x reference)
"""Multi-head attention (S=2048, B=2, D=1024, H=16, Hd=64) on 8 trn2 cores.

Sharding: core = (batch b, head-group g of 4 heads) -> 2*4 = 8 cores.
Each core computes full attention for its 4 heads / 1 batch and a partial
output projection (row-parallel Wo); the host sums the 4 partials per batch
and adds bo.

v2 design (vs the 490us baseline):
  - 8 attention rounds of (head-pair p, 512-col s-quarter): score psum
    pipeline 3 deep (3x2 banks) + 2 chain accumulators = 8 banks.
  - scores row-group packed (2 heads concurrent, K=64 at rows 0/64).
  - exp split across engines: most tiles exact on ACT (exp with bias
    ln(C_EFF) to match scales), a few per round on DVE via a 2-term
    Schraudolph bit-trick exp (f32->int16 tensor_scalar passes, bitcast to
    bf16, summed on GpSimd) -- relative error ~ +-1.5%, softmax-normalized.
  - chains drained psum->sbuf by ACT right after the last accumulate
    (frees psum for the next round); Z row kept f32; DRAM-bounce partition
    broadcast of Z off the critical path; DVE reciprocal + normalize mults.
  - attn2 / Wo in bf16; K-outer projections start on first DMA'd x tile;
    out-projection bursts after each s-quarter completes both pairs.
"""

import sys

for _p in ("/opt/trn_rl_repo", "/root/.axon_site/_ro/trn_rl_repo"):
    if _p not in sys.path:
        sys.path.insert(0, _p)

import numpy as np
import ml_dtypes

S = 2048
B = 2
D = 1024
H = 16
HD = 64
NH = 4   # heads per core
P = 128
KD = D // P   # 8 contraction tiles for projections
NT = S // P   # 16 t (key) tiles
NQ = 4        # s-quarters per pair
QW = S // NQ  # 512 columns per quarter

BF16 = ml_dtypes.bfloat16

# Schraudolph 2-term exp constants (see accuracy sim):
#   S(y,d) = bitcast_bf16(int16(rint(128*y + 16256 + d)))
#   S(y,-80.25) + S(y,-142.75) ~= C_EFF * 2^y, max rel dev ~1.45%
# with y = score * 0.125 * log2(e).  ACT tiles use exp(0.125*x + ln(C_EFF))
# so both shares carry the same scale; softmax normalization removes it.
LOG2E = 1.4426950408889634
TS_MULT = 0.125 * 128 * LOG2E        # 23.083120654223414
TS_OFF1 = 16256.0 - 80.25
TS_OFF2 = 16256.0 - 142.75
ACT_BIAS = 0.1396463666  # ln(1.1498665502852918)

# which exp units (t index 0..15) go to the DVE path, per round
DVE_T = (1, 5, 9, 12)

_BUILD_CACHE = {}


def build_bass(s=S, dve_t=DVE_T, debug_taps=False):
    import concourse.bacc as bacc
    import concourse.bass as bass
    import concourse.mybir as mybir
    import concourse.tile as tile

    f32 = mybir.dt.float32
    bf16 = mybir.dt.bfloat16
    i16 = mybir.dt.int16
    AF = mybir.ActivationFunctionType
    ALU = mybir.AluOpType

    nt = s // P
    nq = s // QW

    nc = bacc.Bacc("TRN2", target_bir_lowering=False, debug=False, num_devices=8)

    xq = nc.dram_tensor("xq_t", [D, s], bf16, kind="ExternalInput").ap()
    xk = nc.dram_tensor("xk_t", [D, s], bf16, kind="ExternalInput").ap()
    xv = nc.dram_tensor("xv_t", [D, s], bf16, kind="ExternalInput").ap()
    wq = nc.dram_tensor("wq_t", [D, 256], bf16, kind="ExternalInput").ap()
    wk = nc.dram_tensor("wk_t", [D, 256], bf16, kind="ExternalInput").ap()
    wv = nc.dram_tensor("wv_t", [D, 256], bf16, kind="ExternalInput").ap()
    wo = nc.dram_tensor("wo_h", [P, 2, D], bf16, kind="ExternalInput").ap()
    bq2 = nc.dram_tensor("bq2", [P, 2], f32, kind="ExternalInput").ap()
    bk2 = nc.dram_tensor("bk2", [P, 2], f32, kind="ExternalInput").ap()
    bv4 = nc.dram_tensor("bv4", [P, 256], f32, kind="ExternalInput").ap()
    out = nc.dram_tensor("out", [s, D], f32, kind="ExternalOutput").ap()

    from contextlib import ExitStack

    with tile.TileContext(nc) as tc, ExitStack() as ctx:
        consts = ctx.enter_context(tc.tile_pool(name="consts", bufs=1))
        persist = ctx.enter_context(tc.tile_pool(name="persist", bufs=1))
        xpool = ctx.enter_context(tc.tile_pool(name="xpool", bufs=24))
        epool = ctx.enter_context(tc.tile_pool(name="epool", bufs=6))
        tpool = ctx.enter_context(tc.tile_pool(name="tpool", bufs=4))
        cdpool = ctx.enter_context(tc.tile_pool(name="cdpool", bufs=4))
        zpool = ctx.enter_context(tc.tile_pool(name="zpool", bufs=2))
        ospool = ctx.enter_context(tc.tile_pool(name="ospool", bufs=3))
        drampool = ctx.enter_context(tc.tile_pool(name="drampool", bufs=2, space="DRAM"))

        # ---- constants + x loads, in consumption order (K, V, Q) ------
        wk_sb = consts.tile([P, KD, 256], bf16, name="wk_sb")
        nc.sync.dma_start(out=wk_sb, in_=wk.rearrange("(k p) e -> p k e", p=P))
        bk_sb = consts.tile([P, 2], f32, name="bk_sb")
        nc.sync.dma_start(out=bk_sb, in_=bk2)

        # ---- persistent activations -----------------------------------
        q2 = persist.tile([P, 2, s], bf16, name="q2")
        k2 = persist.tile([P, 2, s], bf16, name="k2")
        v_aug = persist.tile([P, NH, nt, 65], bf16, name="v_aug")
        nc.vector.memset(v_aug, 1.0)  # col 64 = ones column -> row 64 is Z
        attn2 = persist.tile([P, 2, s], bf16, name="attn2")

        # ---- projections (k-outer; PE starts on first x tile) ---------
        def load_x(xdram, tag):
            x3 = xdram.rearrange("(k p) s -> k p s", p=P)
            tiles = []
            for k in range(KD):
                xt = xpool.tile([P, s], bf16, tag="x", name=f"{tag}{k}")
                nc.sync.dma_start(out=xt, in_=x3[k])
                tiles.append(xt)
            return tiles

        xk_t = load_x(xk, "xk")

        wq_sb = consts.tile([P, KD, 256], bf16, name="wq_sb")
        nc.sync.dma_start(out=wq_sb, in_=wq.rearrange("(k p) e -> p k e", p=P))
        bq_sb = consts.tile([P, 2], f32, name="bq_sb")
        nc.sync.dma_start(out=bq_sb, in_=bq2)
        xq_t = load_x(xq, "xq")

        wv_sb = consts.tile([P, KD, 256], bf16, name="wv_sb")
        nc.sync.dma_start(out=wv_sb, in_=wv.rearrange("(k p) e -> p k e", p=P))
        bv_sb = consts.tile([P, 256], f32, name="bv_sb")
        nc.sync.dma_start(out=bv_sb, in_=bv4)
        xv_t = load_x(xv, "xv")

        wo_sb = consts.tile([P, 2, D], bf16, name="wo_sb")
        nc.sync.dma_start(out=wo_sb, in_=wo)
        actb = consts.tile([P, 1], f32, name="actb")
        nc.vector.memset(actb, ACT_BIAS)

        def qk_chunk(pp, xt, w_sb, b_sb, dst, accums):
            # accums: list of (p, sh); k-outer over the given accumulators
            ps = {}
            for (p, sh) in accums:
                ps[(p, sh)] = pp.tile([P, s // 2], f32, tag="qk",
                                      name=f"ps{p}{sh}")
            for k in range(KD):
                for (p, sh) in accums:
                    for c in range(2):
                        nc.tensor.matmul(
                            ps[(p, sh)][:, c * QW:(c + 1) * QW],
                            lhsT=w_sb[:, k, p * P:(p + 1) * P],
                            rhs=xt[k][:, sh * (s // 2) + c * QW:
                                      sh * (s // 2) + (c + 1) * QW],
                            start=(k == 0),
                            stop=(k == KD - 1),
                        )
            for (p, sh) in accums:
                nc.vector.tensor_scalar(
                    dst[:, p, sh * (s // 2):(sh + 1) * (s // 2)],
                    ps[(p, sh)], b_sb[:, p:p + 1],
                    None, ALU.add,
                )

        with tc.tile_pool(name="ppqk", bufs=4, space="PSUM") as pp:
            # K then Q (all 4 accumulators each, k-outer)
            qk_chunk(pp, xk_t, wk_sb, bk_sb, k2,
                     [(p, sh) for p in range(2) for sh in range(2)])
            qk_chunk(pp, xq_t, wq_sb, bq_sb, q2,
                     [(p, sh) for p in range(2) for sh in range(2)])
        with tc.tile_pool(name="ppv", bufs=3, space="PSUM") as ppv:
            # V: per-t accumulate (k inner), one bias op into 4 head slices
            for t in range(nt):
                vps = ppv.tile([P, 256], f32, tag="v", name=f"vps{t}")
                for k in range(KD):
                    nc.tensor.matmul(
                        vps,
                        lhsT=xv_t[k][:, t * P:(t + 1) * P],
                        rhs=wv_sb[:, k, :],
                        start=(k == 0),
                        stop=(k == KD - 1),
                    )
                nc.vector.tensor_tensor(
                    v_aug[:, :, t, 0:64],
                    vps.rearrange("p (h d) -> p h d", h=NH),
                    bv_sb.rearrange("p (h d) -> p h d", h=NH),
                    ALU.add,
                )

        # ---- attention rounds + interleaved output projection ---------
        wide = ctx.enter_context(tc.tile_pool(name="wide", bufs=3, space="PSUM"))
        accp = ctx.enter_context(tc.tile_pool(name="accp", bufs=2, space="PSUM"))

        def exp_act(et, sc):
            nc.scalar.activation(et, sc, AF.Exp, bias=actb, scale=0.125)

        def exp_dve(et, sc):
            # half-tile (per-head) passes to cut latency; adds on DVE 4x mode
            t1 = tpool.tile([P, QW * 2], i16, tag="ts", name="t1")
            t2 = tpool.tile([P, QW * 2], i16, tag="ts", name="t2")
            b1 = t1.bitcast(bf16)
            b2 = t2.bitcast(bf16)
            for c in range(2):
                h = slice(c * QW, (c + 1) * QW)
                nc.vector.tensor_scalar(t1[:, h], sc[:, h], TS_MULT, TS_OFF1,
                                        ALU.mult, ALU.add)
                nc.vector.tensor_scalar(t2[:, h], sc[:, h], TS_MULT, TS_OFF2,
                                        ALU.mult, ALU.add)
                nc.gpsimd.tensor_tensor(et[:, h], b1[:, h], b2[:, h], ALU.add)

        # attnV accumulation into psum is commutative: emit each attnV pair a
        # few score-slots after its exp so slow tiles never head-of-line
        # block the in-order PE.  ACT exp ~1us -> shift 2; DVE ~2-3us -> 5.
        SHIFT_ACT, SHIFT_DVE = 2, 7

        def attn_round(p, q):
            cols = slice(q * QW, (q + 1) * QW)
            heads = (2 * p, 2 * p + 1)
            chains = [accp.tile([P, QW], f32, tag="chain", name=f"ch{hi}")
                      for hi in range(2)]
            ets = {}
            emitted = []
            pending = []  # (due_slot, t)

            def emit_av(t, is_first, is_last):
                for hi in range(2):
                    nc.tensor.matmul(
                        chains[hi][0:65, :],
                        lhsT=v_aug[:, heads[hi], t, :],
                        rhs=ets[t][:, hi * QW:(hi + 1) * QW],
                        start=is_first,
                        stop=is_last,
                    )

            for t in range(nt):
                sc = wide.tile([P, 2 * QW], f32, tag="wide", name="sc")
                for hi in range(2):
                    rlo = 64 * hi
                    nc.tensor.matmul(
                        sc[:, hi * QW:(hi + 1) * QW],
                        lhsT=k2[rlo:rlo + 64, p, t * P:(t + 1) * P],
                        rhs=q2[rlo:rlo + 64, p, cols],
                        start=True,
                        stop=True,
                        tile_position=(rlo, 0),
                    )
                et = epool.tile([P, 2 * QW], bf16, tag="exp", name="et")
                ets[t] = et
                if t in dve_t:
                    exp_dve(et, sc)
                    pending.append((t + SHIFT_DVE, t))
                else:
                    exp_act(et, sc)
                    pending.append((t + SHIFT_ACT, t))
                for due, tt_ in [x for x in pending if x[0] <= t]:
                    emit_av(tt_, not emitted, False)
                    emitted.append(tt_)
                    pending.remove((due, tt_))
            pending.sort()
            for i, (due, tt_) in enumerate(pending):
                emit_av(tt_, not emitted, i == len(pending) - 1)
                emitted.append(tt_)
            # drain chains to sbuf (frees psum), Z row separately in f32
            cd = []
            zrow = zpool.tile([1, 2 * QW], f32, tag="zrow", name="zrow")
            for hi in range(2):
                c = cdpool.tile([64, QW], bf16, tag="cd", name=f"cd{hi}")
                nc.scalar.copy(c, chains[hi][0:64, :])
                nc.scalar.copy(zrow[:, hi * QW:(hi + 1) * QW],
                               chains[hi][64:65, :])
                cd.append(c)
            # partition-broadcast 1/Z via DRAM bounce + reciprocal
            zd = drampool.tile([1, 2 * QW], f32, tag="zd", name="zd")
            nc.sync.dma_start(out=zd, in_=zrow)
            zbc = bass.AP(
                tensor=zd.tensor,
                offset=zd.offset,
                ap=[[0, 64]] + list(zd.ap[-1:]),
            )
            rz = zpool.tile([64, 2 * QW], f32, tag="rz", name="rz")
            nc.sync.dma_start(out=rz, in_=zbc)
            nc.vector.reciprocal_approx_fast(rz, rz)
            # normalize (all-SBUF -> gpsimd): even head direct, odd head via
            # DMA row shift
            nc.gpsimd.tensor_tensor(
                attn2[0:64, p, cols], cd[0], rz[:, 0:QW], ALU.mult,
            )
            atmp = zpool.tile([64, QW], bf16, tag="atmp", name="atmp")
            nc.gpsimd.tensor_tensor(atmp, cd[1], rz[:, QW:2 * QW], ALU.mult)
            nc.sync.dma_start(out=attn2[64:128, p, cols], in_=atmp)

        def out_proj(sc_i, drain_eng):
            op = wide.tile([P, D], f32, tag="wide", name="op")
            for nh_i in range(2):
                for p in range(2):
                    nc.tensor.matmul(
                        op[:, nh_i * 512:(nh_i + 1) * 512],
                        lhsT=attn2[:, p, sc_i * P:(sc_i + 1) * P],
                        rhs=wo_sb[:, p, nh_i * 512:(nh_i + 1) * 512],
                        start=(p == 0),
                        stop=(p == 1),
                    )
            ob = ospool.tile([P, D], f32, tag="ob", name="ob")
            if drain_eng == "act":
                nc.scalar.copy(ob, op)
            else:
                nc.vector.tensor_copy(ob, op)
            nc.sync.dma_start(out=out[sc_i * P:(sc_i + 1) * P, :], in_=ob)

        # out-proj burst for quarter q is emitted one round after both its
        # pairs finish, so the odd-row DMA shift is off the critical path
        blks = s // P // nq

        def burst(q):
            for blk in range(blks):
                sc_i = q * blks + blk
                out_proj(sc_i, "dve")

        for q in range(nq):
            attn_round(0, q)
            if q > 0:
                burst(q - 1)
            attn_round(1, q)
        burst(nq - 1)

        if debug_taps:
            dq2 = nc.dram_tensor("dbg_q2", [P, 2, s], bf16, kind="ExternalOutput").ap()
            nc.sync.dma_start(out=dq2, in_=q2)
            dk2 = nc.dram_tensor("dbg_k2", [P, 2, s], bf16, kind="ExternalOutput").ap()
            nc.sync.dma_start(out=dk2, in_=k2)
            dva = nc.dram_tensor("dbg_vaug", [P, NH, nt, 65], bf16, kind="ExternalOutput").ap()
            nc.sync.dma_start(out=dva, in_=v_aug)
            dat = nc.dram_tensor("dbg_attn", [P, 2, s], bf16, kind="ExternalOutput").ap()
            nc.sync.dma_start(out=dat, in_=attn2)

    nc.compile()
    return nc


def get_bass(s=S):
    if s not in _BUILD_CACHE:
        _BUILD_CACHE[s] = build_bass(s)
    return _BUILD_CACHE[s]


def make_in_maps(query, key, value, Wq, bq, Wk, bk, Wv, bv, Wo):
    """Host-side sharding: per-core input dict for core = b*4 + g."""
    in_maps = []
    for core in range(8):
        b, g = core // 4, core % 4
        cs = slice(g * 256, (g + 1) * 256)
        # pair-packed: wo_h[hd + 64*(h%2), h//2, :] = Wo[:, g*256 + h*64 + hd]
        wo_h = (
            np.ascontiguousarray(Wo[:, cs].T)
            .reshape(2, P, D)
            .transpose(1, 0, 2)
        )
        m = {
            "xq_t": np.ascontiguousarray(query[:, b, :].T).astype(BF16),
            "xk_t": np.ascontiguousarray(key[:, b, :].T).astype(BF16),
            "xv_t": np.ascontiguousarray(value[:, b, :].T).astype(BF16),
            "wq_t": np.ascontiguousarray(Wq[cs, :].T).astype(BF16),
            "wk_t": np.ascontiguousarray(Wk[cs, :].T).astype(BF16),
            "wv_t": np.ascontiguousarray(Wv[cs, :].T).astype(BF16),
            "wo_h": np.ascontiguousarray(wo_h).astype(BF16),
            "bq2": np.ascontiguousarray(bq[cs].reshape(2, P).T).astype(np.float32),
            "bk2": np.ascontiguousarray(bk[cs].reshape(2, P).T).astype(np.float32),
            "bv4": np.ascontiguousarray(
                np.broadcast_to(bv[cs], (P, 256))
            ).astype(np.float32),
        }
        in_maps.append(m)
    return in_maps


def kernel(query, key, value, Wq, bq, Wk, bk, Wv, bv, Wo, bo):
    from concourse.bass_utils import run_bass_kernel_spmd

    query = np.asarray(query, dtype=np.float32)
    key = np.asarray(key, dtype=np.float32)
    value = np.asarray(value, dtype=np.float32)
    Wq = np.asarray(Wq, dtype=np.float32)
    Wk = np.asarray(Wk, dtype=np.float32)
    Wv = np.asarray(Wv, dtype=np.float32)
    Wo = np.asarray(Wo, dtype=np.float32)

    nc = get_bass(S)
    in_maps = make_in_maps(query, key, value, Wq, bq, Wk, bk, Wv, bv, Wo)
    res = run_bass_kernel_spmd(nc, in_maps, core_ids=list(range(8)))
    outs = [res.results[c]["out"] for c in range(8)]

    full = np.empty((S, B, D), dtype=np.float32)
    bo32 = np.asarray(bo, dtype=np.float32)
    for b in range(B):
        acc = outs[b * 4].astype(np.float32).copy()
        for g in range(1, 4):
            acc += outs[b * 4 + g]
        full[:, b, :] = acc + bo32[None, :]
    return full
